# revision 5
# baseline (speedup 1.0000x reference)
"""Trainium2 Bass kernel for nn_DifferentialRenderLoss.

Algorithm: the volume-render trilinear gather is separable per depth sample
(rays are axis-aligned: R == I).  For depth sample k the rendered frame is
  out = A_k @ ((1-fz) vol[z0] + fz vol[z1]) @ B_k^T
with A_k [PH,VH], B_k [PW,VW] sparse tent-weight matrices (<=2 nnz/row).
Only samples whose z lies inside the volume contribute (~4 of 200); the
rest multiply exact 1.0 factors into the raymarch and add exact 0.0 to the
sums, so skipping them is lossless.  The z-blend folds into A (two
PSUM-accumulated matmuls).  Each of the 8 cores renders an 18-pixel-wide
column stripe of all cameras (loading only the volume W-slices it needs),
does the raymarch + Huber losses for its stripe, plus a W-chunk of the BEV
reduction, and writes partial sums; the host combines them.
"""
import sys

if "/opt/trn_rl_repo" not in sys.path:
    sys.path.insert(0, "/opt/trn_rl_repo")

import numpy as np

# ---- problem configuration (mirrors the nn.Module init_kwargs) ----
N_CAM = 2
PH, PW = 96, 144
NPTS = 200
MIN_DEPTH, MAX_DEPTH = 1.0, 4000.0
VD, VH, VW = 32, 128, 384
VOXEL = 2.5
VOL_TRANS = np.zeros(3, np.float32)
CH = 4                      # density + rgb
N_CORES = 8
PCW = PW // N_CORES         # pixel columns per core
WB = VW // N_CORES          # bev W-chunk per core


# ---------------------------------------------------------------- host math
def _tent_matrix(g, n):
    """Dense interpolation matrix mirroring the reference's floor/frac +
    per-corner mask + clip arithmetic bitwise (all float32)."""
    P = g.shape[0]
    A = np.zeros((P, n), np.float32)
    g0 = np.floor(g)
    f = (g - g0).astype(np.float32)
    i0 = g0.astype(np.int32)
    rows = np.arange(P)
    for d, w in ((0, (np.float32(1.0) - f).astype(np.float32)), (1, f)):
        idx = i0 + d
        valid = (idx >= 0) & (idx < n)
        np.add.at(A, (rows, np.clip(idx, 0, n - 1)),
                  np.where(valid, w, np.float32(0.0)).astype(np.float32))
    return A


def _plan(focal, principal, R, T):
    """Per-camera active depth samples with tent matrices (float32 host math
    mirroring the reference)."""
    focal = np.asarray(focal, np.float32)
    principal = np.asarray(principal, np.float32)
    R = np.asarray(R, np.float32)
    T = np.asarray(T, np.float32)
    xs = np.arange(PW, dtype=np.float32) + np.float32(0.5)
    ys = np.arange(PH, dtype=np.float32) + np.float32(0.5)
    depths = np.linspace(MIN_DEPTH, MAX_DEPTH, NPTS, dtype=np.float32)
    half = np.array([VOXEL * (VW - 1) / 2.0, VOXEL * (VH - 1) / 2.0,
                     VOXEL * (VD - 1) / 2.0], np.float32)
    cams = []
    for c in range(N_CAM):
        if not np.allclose(R[c], np.eye(3), atol=1e-6):
            raise NotImplementedError("kernel fast path requires R == I")
        dir_x = ((xs - principal[c, 0]) / focal[c, 0]).astype(np.float32)
        dir_y = ((ys - principal[c, 1]) / focal[c, 1]).astype(np.float32)
        origin = (-(T[c] @ R[c].T)).astype(np.float32)
        samples = []
        for k in range(NPTS):
            t = depths[k]
            zw = np.float32(origin[2] + t)          # dir_z == 1
            gz = np.float32((np.float32((zw - VOL_TRANS[2]) / half[2]) + np.float32(1.0))
                            * np.float32(0.5) * (VD - 1))
            if gz <= -1.0 or gz >= VD:
                continue
            z0 = int(np.floor(gz))
            fz = np.float32(gz - np.floor(gz))
            wz0 = np.float32(1.0) - fz if z0 >= 0 else np.float32(0.0)
            wz1 = fz if z0 + 1 <= VD - 1 else np.float32(0.0)
            gy = ((((origin[1] + t * dir_y) - VOL_TRANS[1]) / half[1]
                   + np.float32(1.0)) * np.float32(0.5) * (VH - 1)).astype(np.float32)
            gx = ((((origin[0] + t * dir_x) - VOL_TRANS[0]) / half[0]
                   + np.float32(1.0)) * np.float32(0.5) * (VW - 1)).astype(np.float32)
            A = _tent_matrix(gy, VH)
            B = _tent_matrix(gx, VW)
            if not (A.any() and B.any() and (wz0 or wz1)):
                continue
            samples.append(dict(k=k, z0=min(max(z0, 0), VD - 1),
                                z1=min(max(z0 + 1, 0), VD - 1),
                                wz0=wz0, wz1=wz1, A=A, B=B))
        cams.append(samples)
    return cams


def _dedup(cams):
    """Group cameras with identical plans. Returns (unique_plans, cam2u)."""
    uniq, cam2u = [], []
    for s in cams:
        found = None
        for ui, u in enumerate(uniq):
            if len(u) == len(s) and all(
                a["k"] == b["k"] and a["z0"] == b["z0"] and a["z1"] == b["z1"]
                and a["wz0"] == b["wz0"] and a["wz1"] == b["wz1"]
                and np.array_equal(a["A"], b["A"]) and np.array_equal(a["B"], b["B"])
                for a, b in zip(u, s)
            ):
                found = ui
                break
        if found is None:
            uniq.append(list(s))
            cam2u.append(len(uniq) - 1)
        else:
            cam2u.append(found)
    return uniq, cam2u


def _pad_plans(uniq):
    """Pad every unique plan to a common NK with all-zero dummy samples
    (zero A/B/wz => exact zero density, raymarch unaffected)."""
    nk = max([len(u) for u in uniq] + [1])
    zero = dict(k=-1, z0=0, z1=0, wz0=np.float32(0), wz1=np.float32(0),
                A=np.zeros((PH, VH), np.float32),
                B=np.zeros((PW, VW), np.float32))
    for u in uniq:
        while len(u) < nk:
            u.append(zero)
    return nk


# ------------------------------------------------------------ device program
_PROG_CACHE = {}


def _build_program(NU, NK, WC, cam2u):
    import concourse.bacc as bacc
    import concourse.mybir as mybir
    import concourse.tile as tile

    F32 = mybir.dt.float32
    AF = mybir.ActivationFunctionType
    AX = mybir.AxisListType
    OP = mybir.AluOpType

    nc = bacc.Bacc(None)
    vol_d = nc.dram_tensor("volpack", [128, NU * NK * 2 * CH * WC], F32,
                           kind="ExternalInput")
    at_d = nc.dram_tensor("atpack", [128, NU * NK * 2 * PH], F32,
                          kind="ExternalInput")
    bt_d = nc.dram_tensor("btpack", [WC, NU * NK * PCW], F32,
                          kind="ExternalInput")
    tgt_d = nc.dram_tensor("tgtpack", [PCW, N_CAM * 4 * PH], F32,
                           kind="ExternalInput")
    bev_d = nc.dram_tensor("bevpack", [128, WB * VD], F32, kind="ExternalInput")
    out_d = nc.dram_tensor("out", [1, 16], F32, kind="ExternalOutput")

    with tile.TileContext(nc) as tc:
        with tc.tile_pool(name="sb", bufs=1) as sb, \
             tc.tile_pool(name="ps1", bufs=2, space="PSUM") as ps1, \
             tc.tile_pool(name="ps2", bufs=2, space="PSUM") as ps2, \
             tc.tile_pool(name="ps3", bufs=1, space="PSUM") as ps3:

            vol_sb = sb.tile([128, NU * NK * 2 * CH * WC], F32)
            at_sb = sb.tile([128, NU * NK * 2 * PH], F32)
            bt_sb = sb.tile([WC, NU * NK * PCW], F32)
            tgt_sb = sb.tile([PCW, N_CAM * 4 * PH], F32)
            bev_sb = sb.tile([128, WB * VD], F32)

            # DMAs: per-(u,k) chunks of the volume pack so compute starts early
            ck = 2 * CH * WC
            for uk in range(NU * NK):
                nc.sync.dma_start(out=vol_sb[:, uk * ck:(uk + 1) * ck],
                                  in_=vol_d[:, uk * ck:(uk + 1) * ck])
            nc.sync.dma_start(out=at_sb[:], in_=at_d[:])
            nc.sync.dma_start(out=bt_sb[:], in_=bt_d[:])
            nc.sync.dma_start(out=tgt_sb[:], in_=tgt_d[:])
            half = WB * VD // 2
            nc.sync.dma_start(out=bev_sb[:, :half], in_=bev_d[:, :half])
            nc.sync.dma_start(out=bev_sb[:, half:], in_=bev_d[:, half:])

            # ---- render: P_sb[u][k] [PCW, CH*PH] (pixel-col x (ch,row)) ----
            P_sb = [[None] * NK for _ in range(NU)]
            for u in range(NU):
                for k in range(NK):
                    uk = u * NK + k
                    y_ps = ps1.tile([WC, CH * PH], F32, tag="y")
                    for ch in range(CH):
                        for zc in (0, 1):
                            voff = (uk * 2 + zc) * CH * WC + ch * WC
                            aoff = (uk * 2 + zc) * PH
                            nc.tensor.matmul(
                                y_ps[:, ch * PH:(ch + 1) * PH],
                                lhsT=vol_sb[:, voff:voff + WC],
                                rhs=at_sb[:, aoff:aoff + PH],
                                start=(zc == 0), stop=(zc == 1))
                    y_sb = sb.tile([WC, CH * PH], F32, tag="ysb")
                    nc.scalar.copy(out=y_sb[:], in_=y_ps[:])
                    p_ps = ps2.tile([PCW, CH * PH], F32, tag="p")
                    nc.tensor.matmul(p_ps[:], lhsT=bt_sb[:, uk * PCW:(uk + 1) * PCW],
                                     rhs=y_sb[:], start=True, stop=True)
                    p_sb = sb.tile([PCW, CH * PH], F32, tag=f"p_{u}_{k}")
                    nc.vector.tensor_copy(out=p_sb[:], in_=p_ps[:])
                    P_sb[u][k] = p_sb

            # ---- raymarch per unique cam ----
            feats = [[None] * 3 for _ in range(NU)]   # [PCW, PH] each
            opac = [None] * NU
            for u in range(NU):
                d0 = P_sb[u][0][:, 0:PH]
                for c in range(3):
                    f_t = sb.tile([PCW, PH], F32, tag=f"feat{u}_{c}_0")
                    nc.vector.tensor_mul(f_t[:], d0,
                                         P_sb[u][0][:, (1 + c) * PH:(2 + c) * PH])
                    feats[u][c] = f_t
                shifted = sb.tile([PCW, PH], F32, tag=f"sh{u}_0")
                nc.vector.tensor_scalar(shifted[:], d0, -1.0, 1.0, OP.mult, OP.add)
                for k in range(1, NK):
                    dk = P_sb[u][k][:, 0:PH]
                    w_t = sb.tile([PCW, PH], F32, tag=f"w{u}_{k}")
                    nc.vector.tensor_mul(w_t[:], dk, shifted[:])
                    for c in range(3):
                        t_t = sb.tile([PCW, PH], F32, tag=f"t{u}_{k}_{c}")
                        nc.vector.tensor_mul(
                            t_t[:], w_t[:], P_sb[u][k][:, (1 + c) * PH:(2 + c) * PH])
                        f_new = sb.tile([PCW, PH], F32, tag=f"feat{u}_{c}_{k}")
                        nc.vector.tensor_add(f_new[:], feats[u][c][:], t_t[:])
                        feats[u][c] = f_new
                    om = sb.tile([PCW, PH], F32, tag=f"om{u}_{k}")
                    nc.vector.tensor_scalar(om[:], dk, -1.0, 1.0, OP.mult, OP.add)
                    sh_new = sb.tile([PCW, PH], F32, tag=f"sh{u}_{k}")
                    nc.vector.tensor_mul(sh_new[:], shifted[:], om[:])
                    shifted = sh_new
                op_t = sb.tile([PCW, PH], F32, tag=f"op{u}")
                nc.vector.tensor_scalar(op_t[:], shifted[:], -1.0, 1.0, OP.mult, OP.add)
                opac[u] = op_t

            # ---- Huber losses per camera: sum(sqrt(1 + 100*diff^2)) ----
            # huber = (sqrt(1+diff^2/0.01)-1)*0.1 summed =>
            #   0.1*(S - count); host applies the affine part.
            acc_cols = []        # (pack column, acc tile)
            for cam in range(N_CAM):
                u = cam2u[cam]
                # silhouette
                diff = sb.tile([PCW, PH], F32, tag=f"sd{cam}")
                nc.vector.tensor_sub(diff[:], opac[u][:],
                                     tgt_sb[:, (cam * 4) * PH:(cam * 4 + 1) * PH])
                sq = sb.tile([PCW, PH], F32, tag=f"ss{cam}")
                nc.vector.tensor_mul(sq[:], diff[:], diff[:])
                hub = sb.tile([PCW, PH], F32, tag=f"sh_hub{cam}")
                acc = sb.tile([PCW, 1], F32, tag=f"acc_s{cam}")
                nc.scalar.activation(hub[:], sq[:], AF.Sqrt, bias=1.0, scale=100.0,
                                     accum_out=acc[:])
                acc_cols.append((cam, acc))
                # color channels
                for c in range(3):
                    dif = sb.tile([PCW, PH], F32, tag=f"cd{cam}_{c}")
                    nc.vector.tensor_sub(
                        dif[:], feats[u][c][:],
                        tgt_sb[:, (cam * 4 + 1 + c) * PH:(cam * 4 + 2 + c) * PH])
                    sqc = sb.tile([PCW, PH], F32, tag=f"cs{cam}_{c}")
                    nc.vector.tensor_mul(sqc[:], dif[:], dif[:])
                    hubc = sb.tile([PCW, PH], F32, tag=f"ch{cam}_{c}")
                    accc = sb.tile([PCW, 1], F32, tag=f"acc_c{cam}_{c}")
                    nc.scalar.activation(hubc[:], sqc[:], AF.Sqrt, bias=1.0,
                                         scale=100.0, accum_out=accc[:])
                    acc_cols.append((N_CAM + cam * 3 + c, accc))

            # ---- bev: sum over (h,w-chunk) of |max_d density| ----
            bmax = sb.tile([128, WB], F32)
            nc.vector.reduce_max(bmax[:], bev_sb[:].rearrange("p (w d) -> p w d", d=VD),
                                 axis=AX.X)
            bsum = sb.tile([128, 1], F32)
            nc.vector.tensor_reduce(bsum[:], bmax[:], axis=AX.X, op=OP.add,
                                    apply_absolute_value=True)

            # ---- cross-partition reduction via ones-matmul ----
            pack = sb.tile([128, 16], F32)
            nc.vector.memset(pack[:], 0.0)
            for col, acc in acc_cols:
                nc.vector.tensor_copy(out=pack[0:PCW, col:col + 1], in_=acc[:])
            nc.vector.tensor_copy(out=pack[:, 15:16], in_=bsum[:])
            ones = sb.tile([128, 1], F32)
            nc.vector.memset(ones[:], 1.0)
            out_ps = ps3.tile([1, 16], F32)
            nc.tensor.matmul(out_ps[:], lhsT=ones[:], rhs=pack[:],
                             start=True, stop=True)
            out_sb = sb.tile([1, 16], F32)
            nc.scalar.copy(out=out_sb[:], in_=out_ps[:])
            nc.sync.dma_start(out=out_d[:], in_=out_sb[:])

    nc.compile()
    return nc


# ------------------------------------------------------------- host packing
def _pack_core(core, uniq, NK, WC, vol, dens, tsil, timg):
    NU = len(uniq)
    qlo = core * PCW
    qhi = qlo + PCW
    # union W-range over all (u,k) for this core's pixel columns
    wlo = VW
    whi = 0
    for u in uniq:
        for s in u:
            cols = np.nonzero(s["B"][qlo:qhi].any(axis=0))[0]
            if cols.size:
                wlo = min(wlo, int(cols[0]))
                whi = max(whi, int(cols[-1]) + 1)
    if wlo >= whi:
        wlo, whi = 0, 1
    if whi - wlo > WC:
        raise AssertionError(f"core {core}: W-range {whi - wlo} > WC {WC}")
    wlo = min(wlo, VW - 1)
    span = min(WC, VW - wlo)

    volpack = np.zeros((128, NU * NK * 2 * CH * WC), np.float32)
    atpack = np.zeros((128, NU * NK * 2 * PH), np.float32)
    btpack = np.zeros((WC, NU * NK * PCW), np.float32)
    for ui, u in enumerate(uniq):
        for k, s in enumerate(u):
            uk = ui * NK + k
            for zc, (z, wz) in enumerate(((s["z0"], s["wz0"]), (s["z1"], s["wz1"]))):
                for ch in range(CH):
                    off = (uk * 2 + zc) * CH * WC + ch * WC
                    volpack[:, off:off + span] = vol[ch, z, :, wlo:wlo + span]
                aoff = (uk * 2 + zc) * PH
                atpack[:, aoff:aoff + PH] = (s["A"] * wz).T
            btpack[:span, uk * PCW:(uk + 1) * PCW] = s["B"][qlo:qhi, wlo:wlo + span].T
    tgtpack = np.zeros((PCW, N_CAM * 4 * PH), np.float32)
    for cam in range(N_CAM):
        tgtpack[:, (cam * 4) * PH:(cam * 4 + 1) * PH] = tsil[cam, :, qlo:qhi].T
        for c in range(3):
            tgtpack[:, (cam * 4 + 1 + c) * PH:(cam * 4 + 2 + c) * PH] = \
                timg[cam, :, qlo:qhi, c].T
    blo = core * WB
    bevpack = np.ascontiguousarray(
        dens[:, :, blo:blo + WB].transpose(1, 2, 0)).reshape(128, WB * VD)
    return dict(volpack=volpack, atpack=atpack, btpack=btpack,
                tgtpack=tgtpack, bevpack=np.ascontiguousarray(bevpack))


def _compute_wc(uniq):
    """Max W-range width over all cores, padded to a multiple of 4."""
    wc = 1
    for core in range(N_CORES):
        qlo, qhi = core * PCW, (core + 1) * PCW
        wlo, whi = VW, 0
        for u in uniq:
            for s in u:
                cols = np.nonzero(s["B"][qlo:qhi].any(axis=0))[0]
                if cols.size:
                    wlo = min(wlo, int(cols[0]))
                    whi = max(whi, int(cols[-1]) + 1)
        if wlo < whi:
            wc = max(wc, whi - wlo)
    wc = min(-(-wc // 4) * 4, 128)
    return wc


# ------------------------------------------------------------------- kernel
_RUN_MODE = "hw"     # "hw" | "sim" (CoreSim, debugging only)


def _run(nc, in_maps):
    if _RUN_MODE == "sim":
        from concourse.bass_interp import CoreSim

        class R:
            results = []
        for m in in_maps:
            sim = CoreSim(nc)
            for name, arr in m.items():
                sim.tensor(name)[:] = arr
            sim.simulate()
            R.results.append({"out": np.array(sim.tensor("out"))})
        return R
    from concourse.bass_utils import run_bass_kernel_spmd
    res = run_bass_kernel_spmd(nc, in_maps, list(range(N_CORES)))
    global _LAST_RESULT
    _LAST_RESULT = res
    return res


_LAST_RESULT = None


def kernel(densities, colors, target_silhouettes, target_images,
           focal, principal, R, T):

    densities = np.asarray(densities, np.float32)
    colors = np.asarray(colors, np.float32)
    tsil = np.asarray(target_silhouettes, np.float32)
    timg = np.asarray(target_images, np.float32)

    cams = _plan(focal, principal, R, T)
    uniq, cam2u = _dedup(cams)
    NK = _pad_plans(uniq)
    NU = len(uniq)
    WC = _compute_wc(uniq)
    assert WC <= 128, f"WC={WC} exceeds PE stationary width"

    key = (NU, NK, WC, tuple(cam2u))
    if key not in _PROG_CACHE:
        _PROG_CACHE[key] = _build_program(NU, NK, WC, cam2u)
    nc = _PROG_CACHE[key]

    vol = np.concatenate([densities[0], colors[0]], axis=0)  # [4,VD,VH,VW]
    dens = densities[0, 0]                                    # [VD,VH,VW]
    in_maps = [_pack_core(c, uniq, NK, WC, vol, dens, tsil, timg)
               for c in range(N_CORES)]
    res = _run(nc, in_maps)

    sil_S = 0.0
    col_S = 0.0
    bev_S = 0.0
    for c in range(N_CORES):
        o = res.results[c]["out"][0]
        sil_S += float(o[0:N_CAM].sum())
        col_S += float(o[N_CAM:N_CAM + 3 * N_CAM].sum())
        bev_S += float(o[15])
    n_sil = N_CAM * PH * PW
    n_col = N_CAM * PH * PW * 3
    sil_err = np.float32(0.1 * (sil_S - n_sil) / n_sil)
    col_err = np.float32(0.1 * (col_S - n_col) / n_col)
    bev_err = np.float32(bev_S / (VH * VW))
    return (col_err, sil_err, bev_err)
